# revision 28
# baseline (speedup 1.0000x reference)
"""Trainium2 Bass/Tile kernel for an attention block:
GroupNorm(32) -> 1x1 conv q/k/v -> softmax attention over 4096 tokens -> 1x1 proj -> +residual.

Sharding: 8 cores = 4 batches x 2 query-halves. Each core receives its batch's
full token set (rolled so its own 2048 query rows come first), computes the
groupnorm stats + full k/v, and attends its 2048 queries against all 4096 keys.

All matmuls run in bf16 on the PE array with f32 PSUM accumulation. The
groupnorm affine is folded into the q/k/v weights (per-channel scale on the
contraction dim) so normalized activations are never materialized; the v-bias
and projection bias fold into one final per-channel vector added with the
residual. Softmax is two-block online: each 2048-key half exponentiates
against its own running max, and the first half is rescaled by
exp(m_A - m) before p @ v.
"""

import numpy as np
from contextlib import ExitStack

import concourse.bass as bass
import concourse.tile as tile
from concourse import bacc, mybir
from concourse.bass_utils import run_bass_kernel_spmd
from concourse.masks import make_identity

B, H, W, C, G = 4, 64, 64, 512, 32
HW = H * W            # 4096 tokens
QH = HW // 2          # 2048 queries per core
P = 128
NT = HW // P          # 32 token tiles
NQ = QH // P          # 16 query blocks per core
NCH = C // P          # 4 channel chunks
GSIZE = C // G        # 16 channels per group
EPS = 1e-5
SC = 1.0 / float(np.sqrt(C))
NTOK = float(HW * GSIZE)  # elements per (batch, group) for stats

FP32 = mybir.dt.float32
BF16 = mybir.dt.bfloat16
FP8 = mybir.dt.float8e4
FP8_ATTN = True           # fp8e4m3 + DoubleRow for scores and attn@v
QSCALE = 8.0              # q is stored as 8*q to keep fp8 operands in range
ESC = SC / QSCALE         # exp() reads raw q*k psum scaled by this
LN_PSCALE = float(np.log(128.0))  # p stored as 128*p in fp8 (e4m3 max=240)
AF = mybir.ActivationFunctionType
ALU = mybir.AluOpType
AX = mybir.AxisListType


def _chunk_major_from_dram(scratch_row):
    """DRAM [1, C] row -> source AP emitting elements in the order that fills
    dest [128, NCH] with dest[p, j] = src[j*128 + p]."""
    return bass.AP(tensor=scratch_row.tensor, offset=scratch_row.offset,
                   ap=[[1, P], [P, NCH]])


def build_program(reps=1):
    nc = bacc.Bacc("TRN2", target_bir_lowering=False, debug=False)
    x_d = nc.dram_tensor("x", [HW, C], FP32, kind="ExternalInput").ap()
    w_d = {n: nc.dram_tensor(n, [C, C], FP32, kind="ExternalInput").ap()
           for n in ("wq", "wk", "wv", "wp")}
    vec_d = {n: nc.dram_tensor(n, [1, C], FP32, kind="ExternalInput").ap()
             for n in ("bq", "bk", "bv", "bp", "gamma", "beta")}
    out_d = nc.dram_tensor("out", [QH, C], FP32, kind="ExternalOutput").ap()
    with tile.TileContext(nc) as tc:
        for _ in range(reps):
            _body(tc, x_d, w_d, vec_d, out_d)
    nc.compile()
    return nc


def _body(tc, x_d, w_d, vec_d, out_d, ablate=()):
    nc = tc.nc
    with ExitStack() as ctx:
        persist = ctx.enter_context(tc.tile_pool(name="persist", bufs=1))
        vecs = ctx.enter_context(tc.tile_pool(name="vecs", bufs=1))
        tiny = ctx.enter_context(tc.tile_pool(name="tiny", bufs=4))
        xf_pool = ctx.enter_context(tc.tile_pool(name="xf", bufs=2))
        xb_pool = ctx.enter_context(tc.tile_pool(name="xb", bufs=3))
        xsq_pool = ctx.enter_context(tc.tile_pool(name="xsq", bufs=2))
        p_pool = ctx.enter_context(tc.tile_pool(name="p", bufs=3))
        pT_pool = ctx.enter_context(tc.tile_pool(name="pT", bufs=2))
        obf_pool = ctx.enter_context(tc.tile_pool(name="obf", bufs=2))
        oT_pool = ctx.enter_context(tc.tile_pool(name="oT", bufs=2))
        xr_pool = ctx.enter_context(tc.tile_pool(name="xr", bufs=2))
        res_pool = ctx.enter_context(tc.tile_pool(name="res", bufs=2))
        tpose_ps = ctx.enter_context(
            tc.tile_pool(name="tpose_ps", bufs=1, space="PSUM"))
        dram_sc = ctx.enter_context(
            tc.tile_pool(name="dram_sc", bufs=1, space="DRAM"))
        scratch = dram_sc.tile([8, C], FP32, tag="scratch")
        _sc_row = [0]

        def chunk_major(dst, src_1xc):
            """dst [128, NCH] <- src [1, C] via a DRAM bounce so that
            dst[p, j] = src[j*128 + p]."""
            r = _sc_row[0]
            _sc_row[0] += 1
            nc.gpsimd.dma_start(scratch[r:r + 1, :], src_1xc)
            nc.gpsimd.dma_start(dst, _chunk_major_from_dram(scratch[r:r + 1, :]))

        # ---- persistent tiles -------------------------------------------
        ident = persist.tile([P, P], BF16, tag="ident")
        make_identity(nc, ident)
        if FP8_ATTN:
            ident8 = persist.tile([P, P], FP8, tag="ident8")
            make_identity(nc, ident8)
        ones = persist.tile([P, 1], BF16, tag="ones")
        nc.vector.memset(ones, 1.0)

        DT_ATT = FP8 if FP8_ATTN else BF16
        xT = persist.tile([P, NCH, HW], BF16, tag="xT")      # x^T (bf16 cast)
        kT = persist.tile([P, NCH, HW], DT_ATT, tag="kT")    # k^T
        qT = persist.tile([P, NCH, QH], DT_ATT, tag="qT")    # q^T (pre-scaled)
        v_sb = persist.tile([P, NT, C], DT_ATT, tag="v")     # v token-major
        w_bf = {n: persist.tile([P, NCH, C], BF16, tag=f"wbf_{n}",
                                name=f"wbf_{n}")
                for n in ("wq", "wk", "wv", "wp")}
        bias_q_t = persist.tile([P, NCH], FP32, tag="bias_q_t")
        bias_k_t = persist.tile([P, NCH], FP32, tag="bias_k_t")
        a_t = persist.tile([P, NCH], FP32, tag="a_t")
        b_t = persist.tile([P, NCH], FP32, tag="b_t")
        bv_t = persist.tile([P, NCH], FP32, tag="bv_t")
        bv_tb = persist.tile([P, NCH], BF16, tag="bv_tb")
        bfin_bc = persist.tile([P, C], FP32, tag="bfin_bc")

        # small [1, C] working vectors
        vget = {}
        for n in ("bq", "bk", "bv", "bp", "gamma", "beta"):
            vget[n] = vecs.tile([1, C], FP32, tag=f"v_{n}", name=f"v_{n}")
            nc.sync.dma_start(vget[n], vec_d[n])

        # weights f32 staging -> bf16 cast
        wstage = ctx.enter_context(tc.tile_pool(name="wstage", bufs=2))
        for n in ("wq", "wk", "wv", "wp"):
            for j in range(NCH):
                wf = wstage.tile([P, C], FP32, tag="wstage")
                nc.sync.dma_start(wf, w_d[n][j * P:(j + 1) * P, :])
                nc.vector.tensor_copy(w_bf[n][:, j, :], wf)

        # q/k biases don't depend on the groupnorm stats (the affine is
        # folded into xT): prep them right away
        bq_sc = vecs.tile([1, C], FP32, tag="bq_sc")
        nc.vector.tensor_scalar_mul(bq_sc, vget["bq"],
                                    QSCALE if FP8_ATTN else SC)
        chunk_major(bias_q_t, bq_sc)
        chunk_major(bias_k_t, vget["bk"])
        chunk_major(bv_t, vget["bv"])
        nc.vector.tensor_copy(bv_tb, bv_t)

        # =================================================================
        # Phase 1: stream x -> stats (sum, sum of squares) + transposed bf16 x
        # =================================================================
        if "p1" in ablate:
            pass
        elif True:
          with tc.tile_pool(name="stats_ps", bufs=1, space="PSUM") as stats_ps:
            sums_ps = stats_ps.tile([1, C], FP32, tag="sums")
            sq_ps = stats_ps.tile([1, C], FP32, tag="sqsums")
            for ti in range(NT):
                xf = xf_pool.tile([P, C], FP32)
                nc.sync.dma_start(xf, x_d[ti * P:(ti + 1) * P, :])
                xb = xb_pool.tile([P, C], BF16)
                nc.vector.tensor_copy(xb, xf)
                xsq = xsq_pool.tile([P, C], BF16)
                nc.scalar.activation(xsq, xb, AF.Square)
                nc.tensor.matmul(sums_ps, ones, xb,
                                 start=(ti == 0), stop=(ti == NT - 1))
                nc.tensor.matmul(sq_ps, ones, xsq,
                                 start=(ti == 0), stop=(ti == NT - 1))
                tp = tpose_ps.tile([P, NCH * P], BF16, tag="tpose")
                for j in range(NCH):
                    nc.tensor.transpose(tp[:, j * P:(j + 1) * P],
                                        xb[:, j * P:(j + 1) * P], ident)
                nc.vector.tensor_copy(
                    xT[:, :, ti * P:(ti + 1) * P],
                    tp.rearrange("p (j t) -> p j t", j=NCH))

            # ---- stats finalize: per-(group) mean/var -> per-channel a, b
            gs1 = vecs.tile([1, G], FP32, tag="gs1")
            nc.vector.reduce_sum(gs1,
                                 sums_ps.rearrange("p (g d) -> p g d", g=G),
                                 axis=AX.X)
            gs2 = vecs.tile([1, G], FP32, tag="gs2")
            nc.vector.reduce_sum(gs2,
                                 sq_ps.rearrange("p (g d) -> p g d", g=G),
                                 axis=AX.X)
            mean_g = vecs.tile([1, G], FP32, tag="mean_g")
            nc.vector.tensor_scalar_mul(mean_g, gs1, 1.0 / NTOK)
            ex2_g = vecs.tile([1, G], FP32, tag="ex2_g")
            nc.vector.tensor_scalar_mul(ex2_g, gs2, 1.0 / NTOK)
            msq_g = vecs.tile([1, G], FP32, tag="msq_g")
            nc.vector.tensor_mul(msq_g, mean_g, mean_g)
            var_g = vecs.tile([1, G], FP32, tag="var_g")
            nc.vector.tensor_sub(var_g, ex2_g, msq_g)
            eps_t = vecs.tile([1, 1], FP32, tag="eps_t")
            nc.vector.memset(eps_t, EPS)
            rstd_g = vecs.tile([1, G], FP32, tag="rstd_g")
            nc.scalar.activation(rstd_g, var_g, AF.Sqrt, bias=eps_t)
            nc.vector.reciprocal(rstd_g, rstd_g)

            a_c = vecs.tile([1, C], FP32, tag="a_c")
            nc.vector.tensor_mul(
                a_c.rearrange("p (g d) -> p g d", g=G),
                rstd_g.to_broadcast([1, G, GSIZE]),
                vget["gamma"].rearrange("p (g d) -> p g d", g=G))
            # b_c = beta - mean_c * a_c
            b_c = vecs.tile([1, C], FP32, tag="b_c")
            nc.vector.tensor_mul(
                b_c.rearrange("p (g d) -> p g d", g=G),
                mean_g.to_broadcast([1, G, GSIZE]),
                a_c.rearrange("p (g d) -> p g d", g=G))
            nc.vector.tensor_sub(b_c, vget["beta"], b_c)

            # rearrange per-channel vectors to per-partition [128, 4] layout
            chunk_major(a_t, a_c)
            chunk_major(b_t, b_c)

            # bfin = bv @ Wp + bp  (added at the very end, post-normalize)
            bfps = stats_ps.tile([1, C], FP32, tag="bf")
            for j in range(NCH):
                nc.tensor.matmul(bfps, bv_tb[:, j:j + 1], w_bf["wp"][:, j, :],
                                 start=(j == 0), stop=(j == NCH - 1))
            bfin = vecs.tile([1, C], FP32, tag="bfin")
            nc.vector.tensor_add(bfin, bfps, vget["bp"])
            nc.gpsimd.partition_broadcast(bfin_bc, bfin)

            # fold the groupnorm affine into xT: xT <- a * xT + b
            for j in range(NCH):
                nc.vector.tensor_scalar(xT[:, j, :], xT[:, j, :],
                                        a_t[:, j:j + 1], b_t[:, j:j + 1],
                                        op0=ALU.mult, op1=ALU.add)

        # =================================================================
        # Phase 2: projections q^T, k^T (channel-major) and v (token-major)
        # =================================================================
        mm_ps = ctx.enter_context(
            tc.tile_pool(name="mm_ps", bufs=5, space="PSUM"))
        out_ps_pool = ctx.enter_context(
            tc.tile_pool(name="out_ps", bufs=1, space="PSUM"))

        for j in range(NCH) if "p2" not in ablate else []:
            for n in range(HW // 512):
                ps = mm_ps.tile([P, 512], FP32, tag="mm")
                for cj in range(NCH):
                    nc.tensor.matmul(
                        ps, w_bf["wk"][:, cj, j * P:(j + 1) * P],
                        xT[:, cj, n * 512:(n + 1) * 512],
                        start=(cj == 0), stop=(cj == NCH - 1))
                nc.scalar.activation(kT[:, j, n * 512:(n + 1) * 512], ps,
                                     AF.Identity, bias=bias_k_t[:, j:j + 1])
            for n in range(QH // 512):
                ps = mm_ps.tile([P, 512], FP32, tag="mm")
                for cj in range(NCH):
                    nc.tensor.matmul(
                        ps, w_bf["wq"][:, cj, j * P:(j + 1) * P],
                        xT[:, cj, n * 512:(n + 1) * 512],
                        start=(cj == 0), stop=(cj == NCH - 1))
                nc.scalar.activation(qT[:, j, n * 512:(n + 1) * 512], ps,
                                     AF.Identity, bias=bias_q_t[:, j:j + 1],
                                     scale=QSCALE if FP8_ATTN else SC)
        for tk in range(NT) if "p2" not in ablate else []:
            ps = mm_ps.tile([P, C], FP32, tag="mm")
            for cj in range(NCH):
                nc.tensor.matmul(ps, xT[:, cj, tk * P:(tk + 1) * P],
                                 w_bf["wv"][:, cj, :],
                                 start=(cj == 0), stop=(cj == NCH - 1))
            nc.vector.tensor_copy(v_sb[:, tk, :], ps)

        # =================================================================
        # Phase 3: attention, 128 queries at a time, software-pipelined so
        # block qi's scores are issued before block qi-1's attention tail.
        # =================================================================
        def emit_scores_softmax(qi):
            qTi = qT[:, :, qi * P:(qi + 1) * P]
            mx = tiny.tile([P, 8], FP32, tag="mx")
            esum = tiny.tile([P, 8], FP32, tag="esum")
            p_sb = p_pool.tile([P, HW], FP8 if FP8_ATTN else BF16)
            negm = [None, None]
            for h in range(2):
                s_chunks = []
                for n in range(4):
                    ps = mm_ps.tile([P, 512], FP32, tag="mm")
                    if FP8_ATTN:
                        ko = (h * 4 + n) * 512
                        for u in range(2):
                            nc.tensor.matmul(
                                ps, qTi[:, 2 * u:2 * u + 2, :],
                                kT[:, 2 * u:2 * u + 2, ko:ko + 512],
                                start=(u == 0), stop=(u == 1),
                                perf_mode=mybir.MatmulPerfMode.DoubleRow)
                    else:
                        for j in range(NCH):
                            nc.tensor.matmul(
                                ps, qTi[:, j, :],
                                kT[:, j, (h * 4 + n) * 512:(h * 4 + n + 1) * 512],
                                start=(j == 0), stop=(j == NCH - 1))
                    if "nomax" not in ablate:
                        nc.vector.reduce_max(mx[:, h * 4 + n:h * 4 + n + 1],
                                             ps, axis=AX.X)
                    s_chunks.append(ps)
                nm = tiny.tile([P, 1], FP32, tag=f"negm{h}")
                if "nomax" in ablate:
                    nc.vector.memset(nm, 0.0)
                else:
                    nc.vector.reduce_max(nm, mx[:, h * 4:h * 4 + 4], axis=AX.X,
                                         negate=True)  # = -max_h (psum units)
                negm[h] = nm
                if h == 1:
                    gnm = tiny.tile([P, 1], FP32, tag="gnegm")
                    nc.vector.tensor_tensor(gnm, negm[0], negm[1],
                                            op=ALU.min)  # = -max(m_A, m_B)
                    negm[1] = gnm
                if FP8_ATTN:
                    # p = exp(ESC*(s - m)) * 256, stored fp8
                    ebias = tiny.tile([P, 1], FP32, tag=f"ebias{h}")
                    nc.vector.tensor_scalar(ebias, negm[h], ESC, LN_PSCALE,
                                            op0=ALU.mult, op1=ALU.add)
                    escale = ESC
                else:
                    ebias = negm[h]
                    escale = 1.0
                for n in range(4):
                    nc.scalar.activation(
                        p_sb[:, (h * 4 + n) * 512:(h * 4 + n + 1) * 512],
                        s_chunks[n], AF.Exp, bias=ebias, scale=escale,
                        accum_out=esum[:, h * 4 + n:h * 4 + n + 1])

            # correction r_A = exp(ESC*(m_A - m)) applied at the A/B
            # combine (keeps the A-half transposes off the max_B chain)
            dA = tiny.tile([P, 1], FP32, tag="dA")
            nc.vector.tensor_sub(dA, negm[1], negm[0])  # = m_A - m <= 0
            rA = tiny.tile([P, 1], FP32, tag="rA")
            nc.scalar.activation(rA, dA, AF.Exp,
                                 scale=ESC if FP8_ATTN else 1.0)

            # S = rA * sum_A + sum_B ; combine weights w_A = rA/S, w_B = 1/S
            sA = tiny.tile([P, 1], FP32, tag="sA")
            nc.vector.reduce_sum(sA, esum[:, 0:4], axis=AX.X)
            sB = tiny.tile([P, 1], FP32, tag="sB")
            nc.vector.reduce_sum(sB, esum[:, 4:8], axis=AX.X)
            stot = tiny.tile([P, 1], FP32, tag="stot")
            nc.vector.tensor_mul(stot, sA, rA)
            nc.vector.tensor_add(stot, stot, sB)
            rS = tiny.tile([P, 1], FP32, tag="rS")
            nc.vector.reciprocal(rS, stot)
            wA = tiny.tile([P, 1], FP32, tag="wA")
            nc.vector.tensor_mul(wA, rA, rS)
            return {"qi": qi, "p_sb": p_sb, "rS": rS, "wA": wA}

        def emit_attn_tail(st):
            qi, p_sb, rS, wA = st["qi"], st["p_sb"], st["rS"], st["wA"]
            # transpose p -> pT (key-major)
            if FP8_ATTN:
                pT = pT_pool.tile([P, NT, P], FP8)
                for g in range(2):
                    tp = tpose_ps.tile([P, 16 * P], FP8, tag="tpose")
                    for t16 in range(16):
                        tk = g * 16 + t16
                        nc.tensor.transpose(tp[:, t16 * P:(t16 + 1) * P],
                                            p_sb[:, tk * P:(tk + 1) * P],
                                            ident8)
                    nc.scalar.copy(
                        pT[:, g * 16:(g + 1) * 16, :],
                        tp.rearrange("p (a b) -> p a b", a=16))
            else:
                pT = pT_pool.tile([P, NT, P], BF16)
                for g in range(4):
                    tp = tpose_ps.tile([P, 8 * P], BF16, tag="tpose")
                    for t8 in range(8):
                        tk = g * 8 + t8
                        nc.tensor.transpose(tp[:, t8 * P:(t8 + 1) * P],
                                            p_sb[:, tk * P:(tk + 1) * P], ident)
                    nc.vector.tensor_copy(
                        pT[:, g * 8:(g + 1) * 8, :],
                        tp.rearrange("p (a b) -> p a b", a=8))

            # attn @ v: separate accumulators per key half, then the
            # normalized combine obf = wA*out_A + rS*out_B
            opsA = out_ps_pool.tile([P, C], FP32, tag="oA")
            opsB = out_ps_pool.tile([P, C], FP32, tag="oB")
            if FP8_ATTN:
                pT2 = pT.rearrange("p (u two) t -> p u two t", two=2)
                v2 = v_sb.rearrange("p (u two) c -> p u two c", two=2)
                half = NT // 4
                for u in range(NT // 2):
                    dst = opsA if u < half else opsB
                    nc.tensor.matmul(dst, pT2[:, u], v2[:, u],
                                     start=(u % half == 0),
                                     stop=(u % half == half - 1),
                                     perf_mode=mybir.MatmulPerfMode.DoubleRow)
            else:
                half = NT // 2
                for tk in range(NT):
                    dst = opsA if tk < half else opsB
                    nc.tensor.matmul(dst, pT[:, tk, :], v_sb[:, tk, :],
                                     start=(tk % half == 0),
                                     stop=(tk % half == half - 1))
            cmA = obf_pool.tile([P, C], FP32, tag="cmA")
            nc.scalar.activation(cmA, opsA, AF.Identity, scale=wA)
            cmB = obf_pool.tile([P, C], FP32, tag="cmB")
            nc.vector.tensor_scalar_mul(cmB, opsB, rS)
            obf = obf_pool.tile([P, C], BF16, tag="obf")
            nc.vector.tensor_add(obf, cmA, cmB)

            # out^T then projection z = out @ Wp
            ot = tpose_ps.tile([P, NCH * P], BF16, tag="tpose")
            for j in range(NCH):
                nc.tensor.transpose(ot[:, j * P:(j + 1) * P],
                                    obf[:, j * P:(j + 1) * P], ident)
            oT = oT_pool.tile([P, NCH, P], BF16)
            nc.vector.tensor_copy(oT, ot.rearrange("p (a b) -> p a b", a=NCH))
            zps = mm_ps.tile([P, C], FP32, tag="mm")
            for j in range(NCH):
                nc.tensor.matmul(zps, oT[:, j, :], w_bf["wp"][:, j, :],
                                 start=(j == 0), stop=(j == NCH - 1))

            # final: z + bfin + x  -> DRAM
            xr = xr_pool.tile([P, C], FP32)
            nc.sync.dma_start(xr, x_d[qi * P:(qi + 1) * P, :])
            res = res_pool.tile([P, C], FP32, tag="res")
            nc.vector.tensor_add(res, zps, bfin_bc)
            nc.gpsimd.tensor_add(res, res, xr)
            nc.sync.dma_start(out_d[qi * P:(qi + 1) * P, :], res)

        prev = None
        for qi in range(NQ) if "p3" not in ablate else []:
            cur = emit_scores_softmax(qi)
            if prev is not None:
                emit_attn_tail(prev)
            prev = cur
        if prev is not None:
            emit_attn_tail(prev)


_NC_CACHE = None


def _get_program():
    global _NC_CACHE
    if _NC_CACHE is None:
        _NC_CACHE = build_program()
    return _NC_CACHE


def kernel(x, gamma, beta, Wq, bq, Wk, bk, Wv, bv, Wp, bp):
    x = np.asarray(x, dtype=np.float32).reshape(B, HW, C)
    f32 = lambda a: np.ascontiguousarray(np.asarray(a, dtype=np.float32))
    row = lambda a: f32(a).reshape(1, C)
    nc = _get_program()
    in_maps = []
    for core in range(8):
        b, off = core // 2, (core % 2) * QH
        xb = x[b]
        x_roll = np.ascontiguousarray(np.concatenate([xb[off:], xb[:off]], axis=0))
        in_maps.append({
            "x": x_roll,
            "wq": f32(Wq), "wk": f32(Wk), "wv": f32(Wv), "wp": f32(Wp),
            "bq": row(bq), "bk": row(bk), "bv": row(bv), "bp": row(bp),
            "gamma": row(gamma), "beta": row(beta),
        })
    res = run_bass_kernel_spmd(nc, in_maps, core_ids=list(range(8)))
    out = np.empty((B, HW, C), np.float32)
    for core in range(8):
        b, off = core // 2, (core % 2) * QH
        out[b, off:off + QH] = res.results[core]["out"]
    return out.reshape(B, H, W, C)


# revision 29
# speedup vs baseline: 1.2423x; 1.2423x over previous
"""Trainium2 Bass/Tile kernel for an attention block:
GroupNorm(32) -> 1x1 conv q/k/v -> softmax attention over 4096 tokens -> 1x1 proj -> +residual.

Sharding: 8 cores = 4 batches x 2 query-halves. Each core receives its batch's
full token set (rolled so its own 2048 query rows come first), computes the
groupnorm stats + full k/v, and attends its 2048 queries against all 4096 keys.

All matmuls run in bf16 on the PE array with f32 PSUM accumulation. The
groupnorm affine is folded into the q/k/v weights (per-channel scale on the
contraction dim) so normalized activations are never materialized; the v-bias
and projection bias fold into one final per-channel vector added with the
residual. Softmax is two-block online: each 2048-key half exponentiates
against its own running max, and the first half is rescaled by
exp(m_A - m) before p @ v.
"""

import numpy as np
from contextlib import ExitStack

import concourse.bass as bass
import concourse.tile as tile
from concourse import bacc, mybir
from concourse.bass_utils import run_bass_kernel_spmd
from concourse.masks import make_identity

B, H, W, C, G = 4, 64, 64, 512, 32
HW = H * W            # 4096 tokens
QH = HW // 2          # 2048 queries per core
P = 128
NT = HW // P          # 32 token tiles
NQ = QH // P          # 16 query blocks per core
NCH = C // P          # 4 channel chunks
GSIZE = C // G        # 16 channels per group
EPS = 1e-5
SC = 1.0 / float(np.sqrt(C))
NTOK = float(HW * GSIZE)  # elements per (batch, group) for stats

FP32 = mybir.dt.float32
BF16 = mybir.dt.bfloat16
FP8 = mybir.dt.float8e4
FP8_ATTN = True           # fp8e4m3 + DoubleRow for scores and attn@v
QSCALE = 8.0              # q is stored as 8*q to keep fp8 operands in range
ESC = SC / QSCALE         # exp() reads raw q*k psum scaled by this
LN_PSCALE = float(np.log(128.0))  # p stored as 128*p in fp8 (e4m3 max=240)
AF = mybir.ActivationFunctionType
ALU = mybir.AluOpType
AX = mybir.AxisListType


def _chunk_major_from_dram(scratch_row):
    """DRAM [1, C] row -> source AP emitting elements in the order that fills
    dest [128, NCH] with dest[p, j] = src[j*128 + p]."""
    return bass.AP(tensor=scratch_row.tensor, offset=scratch_row.offset,
                   ap=[[1, P], [P, NCH]])


def build_program(reps=1):
    nc = bacc.Bacc("TRN2", target_bir_lowering=False, debug=False)
    x_d = nc.dram_tensor("x", [HW, C], FP32, kind="ExternalInput").ap()
    w_d = {n: nc.dram_tensor(n, [C, C], FP32, kind="ExternalInput").ap()
           for n in ("wq", "wk", "wv", "wp")}
    vec_d = {n: nc.dram_tensor(n, [1, C], FP32, kind="ExternalInput").ap()
             for n in ("bq", "bk", "bv", "bp", "gamma", "beta")}
    out_d = nc.dram_tensor("out", [QH, C], FP32, kind="ExternalOutput").ap()
    with tile.TileContext(nc) as tc:
        for _ in range(reps):
            _body(tc, x_d, w_d, vec_d, out_d)
    nc.compile()
    return nc


def _body(tc, x_d, w_d, vec_d, out_d, ablate=()):
    nc = tc.nc
    with ExitStack() as ctx:
        persist = ctx.enter_context(tc.tile_pool(name="persist", bufs=1))
        vecs = ctx.enter_context(tc.tile_pool(name="vecs", bufs=1))
        tiny = ctx.enter_context(tc.tile_pool(name="tiny", bufs=4))
        xf_pool = ctx.enter_context(tc.tile_pool(name="xf", bufs=2))
        xb_pool = ctx.enter_context(tc.tile_pool(name="xb", bufs=3))
        xsq_pool = ctx.enter_context(tc.tile_pool(name="xsq", bufs=2))
        p_pool = ctx.enter_context(tc.tile_pool(name="p", bufs=3))
        pT_pool = ctx.enter_context(tc.tile_pool(name="pT", bufs=2))
        obf_pool = ctx.enter_context(tc.tile_pool(name="obf", bufs=2))
        oT_pool = ctx.enter_context(tc.tile_pool(name="oT", bufs=2))
        xr_pool = ctx.enter_context(tc.tile_pool(name="xr", bufs=2))
        res_pool = ctx.enter_context(tc.tile_pool(name="res", bufs=2))
        tpose_ps = ctx.enter_context(
            tc.tile_pool(name="tpose_ps", bufs=2, space="PSUM"))
        dram_sc = ctx.enter_context(
            tc.tile_pool(name="dram_sc", bufs=1, space="DRAM"))
        scratch = dram_sc.tile([8, C], FP32, tag="scratch")
        _sc_row = [0]

        def chunk_major(dst, src_1xc):
            """dst [128, NCH] <- src [1, C] via a DRAM bounce so that
            dst[p, j] = src[j*128 + p]."""
            r = _sc_row[0]
            _sc_row[0] += 1
            nc.gpsimd.dma_start(scratch[r:r + 1, :], src_1xc)
            nc.gpsimd.dma_start(dst, _chunk_major_from_dram(scratch[r:r + 1, :]))

        # ---- persistent tiles -------------------------------------------
        ident = persist.tile([P, P], BF16, tag="ident")
        make_identity(nc, ident)
        if FP8_ATTN:
            ident8 = persist.tile([P, P], FP8, tag="ident8")
            make_identity(nc, ident8)
        ones = persist.tile([P, 1], BF16, tag="ones")
        nc.vector.memset(ones, 1.0)

        DT_ATT = FP8 if FP8_ATTN else BF16
        xT = persist.tile([P, NCH, HW], BF16, tag="xT")      # x^T (bf16 cast)
        kT = persist.tile([P, NCH, HW], DT_ATT, tag="kT")    # k^T
        qT = persist.tile([P, NCH, QH], DT_ATT, tag="qT")    # q^T (pre-scaled)
        v_sb = persist.tile([P, NT, C], DT_ATT, tag="v")     # v token-major
        w_bf = {n: persist.tile([P, NCH, C], BF16, tag=f"wbf_{n}",
                                name=f"wbf_{n}")
                for n in ("wq", "wk", "wv", "wp")}
        bias_q_t = persist.tile([P, NCH], FP32, tag="bias_q_t")
        bias_k_t = persist.tile([P, NCH], FP32, tag="bias_k_t")
        a_t = persist.tile([P, NCH], FP32, tag="a_t")
        b_t = persist.tile([P, NCH], FP32, tag="b_t")
        bv_t = persist.tile([P, NCH], FP32, tag="bv_t")
        bv_tb = persist.tile([P, NCH], BF16, tag="bv_tb")
        bfin_bc = persist.tile([P, C], FP32, tag="bfin_bc")

        # small [1, C] working vectors
        vget = {}
        for n in ("bq", "bk", "bv", "bp", "gamma", "beta"):
            vget[n] = vecs.tile([1, C], FP32, tag=f"v_{n}", name=f"v_{n}")
            nc.sync.dma_start(vget[n], vec_d[n])

        # weights f32 staging -> bf16 cast
        wstage = ctx.enter_context(tc.tile_pool(name="wstage", bufs=2))
        for n in ("wq", "wk", "wv", "wp"):
            for j in range(NCH):
                wf = wstage.tile([P, C], FP32, tag="wstage")
                nc.sync.dma_start(wf, w_d[n][j * P:(j + 1) * P, :])
                nc.vector.tensor_copy(w_bf[n][:, j, :], wf)

        # q/k biases don't depend on the groupnorm stats (the affine is
        # folded into xT): prep them right away
        bq_sc = vecs.tile([1, C], FP32, tag="bq_sc")
        nc.vector.tensor_scalar_mul(bq_sc, vget["bq"],
                                    QSCALE if FP8_ATTN else SC)
        chunk_major(bias_q_t, bq_sc)
        chunk_major(bias_k_t, vget["bk"])
        chunk_major(bv_t, vget["bv"])
        nc.vector.tensor_copy(bv_tb, bv_t)

        # =================================================================
        # Phase 1: stream x -> stats (sum, sum of squares) + transposed bf16 x
        # =================================================================
        if "p1" in ablate:
            pass
        elif True:
          with tc.tile_pool(name="stats_ps", bufs=1, space="PSUM") as stats_ps:
            sums_ps = stats_ps.tile([1, C], FP32, tag="sums")
            sq_ps = stats_ps.tile([1, C], FP32, tag="sqsums")
            for ti in range(NT):
                xf = xf_pool.tile([P, C], FP32)
                nc.sync.dma_start(xf, x_d[ti * P:(ti + 1) * P, :])
                xb = xb_pool.tile([P, C], BF16)
                nc.vector.tensor_copy(xb, xf)
                xsq = xsq_pool.tile([P, C], BF16)
                nc.scalar.activation(xsq, xb, AF.Square)
                nc.tensor.matmul(sums_ps, ones, xb,
                                 start=(ti == 0), stop=(ti == NT - 1))
                nc.tensor.matmul(sq_ps, ones, xsq,
                                 start=(ti == 0), stop=(ti == NT - 1))
                tp = tpose_ps.tile([P, NCH * P], BF16, tag="tpose")
                for j in range(NCH):
                    nc.tensor.transpose(tp[:, j * P:(j + 1) * P],
                                        xb[:, j * P:(j + 1) * P], ident)
                nc.vector.tensor_copy(
                    xT[:, :, ti * P:(ti + 1) * P],
                    tp.rearrange("p (j t) -> p j t", j=NCH))

            # ---- stats finalize: per-(group) mean/var -> per-channel a, b
            gs1 = vecs.tile([1, G], FP32, tag="gs1")
            nc.vector.reduce_sum(gs1,
                                 sums_ps.rearrange("p (g d) -> p g d", g=G),
                                 axis=AX.X)
            gs2 = vecs.tile([1, G], FP32, tag="gs2")
            nc.vector.reduce_sum(gs2,
                                 sq_ps.rearrange("p (g d) -> p g d", g=G),
                                 axis=AX.X)
            mean_g = vecs.tile([1, G], FP32, tag="mean_g")
            nc.vector.tensor_scalar_mul(mean_g, gs1, 1.0 / NTOK)
            ex2_g = vecs.tile([1, G], FP32, tag="ex2_g")
            nc.vector.tensor_scalar_mul(ex2_g, gs2, 1.0 / NTOK)
            msq_g = vecs.tile([1, G], FP32, tag="msq_g")
            nc.vector.tensor_mul(msq_g, mean_g, mean_g)
            var_g = vecs.tile([1, G], FP32, tag="var_g")
            nc.vector.tensor_sub(var_g, ex2_g, msq_g)
            eps_t = vecs.tile([1, 1], FP32, tag="eps_t")
            nc.vector.memset(eps_t, EPS)
            rstd_g = vecs.tile([1, G], FP32, tag="rstd_g")
            nc.scalar.activation(rstd_g, var_g, AF.Sqrt, bias=eps_t)
            nc.vector.reciprocal(rstd_g, rstd_g)

            a_c = vecs.tile([1, C], FP32, tag="a_c")
            nc.vector.tensor_mul(
                a_c.rearrange("p (g d) -> p g d", g=G),
                rstd_g.to_broadcast([1, G, GSIZE]),
                vget["gamma"].rearrange("p (g d) -> p g d", g=G))
            # b_c = beta - mean_c * a_c
            b_c = vecs.tile([1, C], FP32, tag="b_c")
            nc.vector.tensor_mul(
                b_c.rearrange("p (g d) -> p g d", g=G),
                mean_g.to_broadcast([1, G, GSIZE]),
                a_c.rearrange("p (g d) -> p g d", g=G))
            nc.vector.tensor_sub(b_c, vget["beta"], b_c)

            # rearrange per-channel vectors to per-partition [128, 4] layout
            chunk_major(a_t, a_c)
            chunk_major(b_t, b_c)

            # bfin = bv @ Wp + bp  (added at the very end, post-normalize)
            bfps = stats_ps.tile([1, C], FP32, tag="bf")
            for j in range(NCH):
                nc.tensor.matmul(bfps, bv_tb[:, j:j + 1], w_bf["wp"][:, j, :],
                                 start=(j == 0), stop=(j == NCH - 1))
            bfin = vecs.tile([1, C], FP32, tag="bfin")
            nc.vector.tensor_add(bfin, bfps, vget["bp"])
            nc.gpsimd.partition_broadcast(bfin_bc, bfin)

            # fold the groupnorm affine into xT: xT <- a * xT + b
            for j in range(NCH):
                nc.vector.tensor_scalar(xT[:, j, :], xT[:, j, :],
                                        a_t[:, j:j + 1], b_t[:, j:j + 1],
                                        op0=ALU.mult, op1=ALU.add)

        # =================================================================
        # Phase 2: projections q^T, k^T (channel-major) and v (token-major)
        # =================================================================
        mm_ps = ctx.enter_context(
            tc.tile_pool(name="mm_ps", bufs=4, space="PSUM"))
        out_ps_pool = ctx.enter_context(
            tc.tile_pool(name="out_ps", bufs=1, space="PSUM"))

        for j in range(NCH) if "p2" not in ablate else []:
            for n in range(HW // 512):
                ps = mm_ps.tile([P, 512], FP32, tag="mm")
                for cj in range(NCH):
                    nc.tensor.matmul(
                        ps, w_bf["wk"][:, cj, j * P:(j + 1) * P],
                        xT[:, cj, n * 512:(n + 1) * 512],
                        start=(cj == 0), stop=(cj == NCH - 1))
                nc.scalar.activation(kT[:, j, n * 512:(n + 1) * 512], ps,
                                     AF.Identity, bias=bias_k_t[:, j:j + 1])
            for n in range(QH // 512):
                ps = mm_ps.tile([P, 512], FP32, tag="mm")
                for cj in range(NCH):
                    nc.tensor.matmul(
                        ps, w_bf["wq"][:, cj, j * P:(j + 1) * P],
                        xT[:, cj, n * 512:(n + 1) * 512],
                        start=(cj == 0), stop=(cj == NCH - 1))
                nc.scalar.activation(qT[:, j, n * 512:(n + 1) * 512], ps,
                                     AF.Identity, bias=bias_q_t[:, j:j + 1],
                                     scale=QSCALE if FP8_ATTN else SC)
        for tk in range(NT) if "p2" not in ablate else []:
            ps = mm_ps.tile([P, C], FP32, tag="mm")
            for cj in range(NCH):
                nc.tensor.matmul(ps, xT[:, cj, tk * P:(tk + 1) * P],
                                 w_bf["wv"][:, cj, :],
                                 start=(cj == 0), stop=(cj == NCH - 1))
            nc.vector.tensor_copy(v_sb[:, tk, :], ps)

        # =================================================================
        # Phase 3: attention, 128 queries at a time, software-pipelined so
        # block qi's scores are issued before block qi-1's attention tail.
        # =================================================================
        def emit_scores_softmax(qi):
            qTi = qT[:, :, qi * P:(qi + 1) * P]
            mx = tiny.tile([P, 8], FP32, tag="mx")
            esum = tiny.tile([P, 8], FP32, tag="esum")
            p_sb = p_pool.tile([P, HW], FP8 if FP8_ATTN else BF16)
            negm = [None, None]
            for h in range(2):
                s_chunks = []
                for n in range(4):
                    ps = mm_ps.tile([P, 512], FP32, tag="mm")
                    if FP8_ATTN:
                        ko = (h * 4 + n) * 512
                        for u in range(2):
                            nc.tensor.matmul(
                                ps, qTi[:, 2 * u:2 * u + 2, :],
                                kT[:, 2 * u:2 * u + 2, ko:ko + 512],
                                start=(u == 0), stop=(u == 1),
                                perf_mode=mybir.MatmulPerfMode.DoubleRow)
                    else:
                        for j in range(NCH):
                            nc.tensor.matmul(
                                ps, qTi[:, j, :],
                                kT[:, j, (h * 4 + n) * 512:(h * 4 + n + 1) * 512],
                                start=(j == 0), stop=(j == NCH - 1))
                    if "nomax" not in ablate:
                        nc.vector.reduce_max(mx[:, h * 4 + n:h * 4 + n + 1],
                                             ps, axis=AX.X)
                    s_chunks.append(ps)
                nm = tiny.tile([P, 1], FP32, tag=f"negm{h}")
                if "nomax" in ablate:
                    nc.vector.memset(nm, 0.0)
                else:
                    nc.vector.reduce_max(nm, mx[:, h * 4:h * 4 + 4], axis=AX.X,
                                         negate=True)  # = -max_h (psum units)
                negm[h] = nm
                if h == 1:
                    gnm = tiny.tile([P, 1], FP32, tag="gnegm")
                    nc.vector.tensor_tensor(gnm, negm[0], negm[1],
                                            op=ALU.min)  # = -max(m_A, m_B)
                    negm[1] = gnm
                if FP8_ATTN:
                    # p = exp(ESC*(s - m)) * 256, stored fp8
                    ebias = tiny.tile([P, 1], FP32, tag=f"ebias{h}")
                    nc.vector.tensor_scalar(ebias, negm[h], ESC, LN_PSCALE,
                                            op0=ALU.mult, op1=ALU.add)
                    escale = ESC
                else:
                    ebias = negm[h]
                    escale = 1.0
                for n in range(4):
                    nc.scalar.activation(
                        p_sb[:, (h * 4 + n) * 512:(h * 4 + n + 1) * 512],
                        s_chunks[n], AF.Exp, bias=ebias, scale=escale,
                        accum_out=esum[:, h * 4 + n:h * 4 + n + 1])

            # correction r_A = exp(ESC*(m_A - m)) applied at the A/B
            # combine (keeps the A-half transposes off the max_B chain)
            dA = tiny.tile([P, 1], FP32, tag="dA")
            nc.vector.tensor_sub(dA, negm[1], negm[0])  # = m_A - m <= 0
            rA = tiny.tile([P, 1], FP32, tag="rA")
            nc.scalar.activation(rA, dA, AF.Exp,
                                 scale=ESC if FP8_ATTN else 1.0)

            # S = rA * sum_A + sum_B ; combine weights w_A = rA/S, w_B = 1/S
            sA = tiny.tile([P, 1], FP32, tag="sA")
            nc.vector.reduce_sum(sA, esum[:, 0:4], axis=AX.X)
            sB = tiny.tile([P, 1], FP32, tag="sB")
            nc.vector.reduce_sum(sB, esum[:, 4:8], axis=AX.X)
            stot = tiny.tile([P, 1], FP32, tag="stot")
            nc.vector.tensor_mul(stot, sA, rA)
            nc.vector.tensor_add(stot, stot, sB)
            rS = tiny.tile([P, 1], FP32, tag="rS")
            nc.vector.reciprocal(rS, stot)
            wA = tiny.tile([P, 1], FP32, tag="wA")
            nc.vector.tensor_mul(wA, rA, rS)
            return {"qi": qi, "p_sb": p_sb, "rS": rS, "wA": wA}

        def emit_attn_tail(st):
            qi, p_sb, rS, wA = st["qi"], st["p_sb"], st["rS"], st["wA"]
            # transpose p -> pT (key-major)
            if FP8_ATTN:
                pT = pT_pool.tile([P, NT, P], FP8)
                for g in range(2):
                    tp = tpose_ps.tile([P, 16 * P], FP8, tag="tpose")
                    for t16 in range(16):
                        tk = g * 16 + t16
                        nc.tensor.transpose(tp[:, t16 * P:(t16 + 1) * P],
                                            p_sb[:, tk * P:(tk + 1) * P],
                                            ident8)
                    nc.scalar.copy(
                        pT[:, g * 16:(g + 1) * 16, :],
                        tp.rearrange("p (a b) -> p a b", a=16))
            else:
                pT = pT_pool.tile([P, NT, P], BF16)
                for g in range(4):
                    tp = tpose_ps.tile([P, 8 * P], BF16, tag="tpose")
                    for t8 in range(8):
                        tk = g * 8 + t8
                        nc.tensor.transpose(tp[:, t8 * P:(t8 + 1) * P],
                                            p_sb[:, tk * P:(tk + 1) * P], ident)
                    nc.vector.tensor_copy(
                        pT[:, g * 8:(g + 1) * 8, :],
                        tp.rearrange("p (a b) -> p a b", a=8))

            # attn @ v: separate accumulators per key half, then the
            # normalized combine obf = wA*out_A + rS*out_B
            opsA = out_ps_pool.tile([P, C], FP32, tag="oA")
            opsB = out_ps_pool.tile([P, C], FP32, tag="oB")
            if FP8_ATTN:
                pT2 = pT.rearrange("p (u two) t -> p u two t", two=2)
                v2 = v_sb.rearrange("p (u two) c -> p u two c", two=2)
                half = NT // 4
                for u in range(NT // 2):
                    dst = opsA if u < half else opsB
                    nc.tensor.matmul(dst, pT2[:, u], v2[:, u],
                                     start=(u % half == 0),
                                     stop=(u % half == half - 1),
                                     perf_mode=mybir.MatmulPerfMode.DoubleRow)
            else:
                half = NT // 2
                for tk in range(NT):
                    dst = opsA if tk < half else opsB
                    nc.tensor.matmul(dst, pT[:, tk, :], v_sb[:, tk, :],
                                     start=(tk % half == 0),
                                     stop=(tk % half == half - 1))
            cmA = obf_pool.tile([P, C], FP32, tag="cmA")
            nc.scalar.activation(cmA, opsA, AF.Identity, scale=wA)
            cmB = obf_pool.tile([P, C], FP32, tag="cmB")
            nc.vector.tensor_scalar_mul(cmB, opsB, rS)
            obf = obf_pool.tile([P, C], BF16, tag="obf")
            nc.vector.tensor_add(obf, cmA, cmB)

            # out^T then projection z = out @ Wp
            ot = tpose_ps.tile([P, NCH * P], BF16, tag="tpose")
            for j in range(NCH):
                nc.tensor.transpose(ot[:, j * P:(j + 1) * P],
                                    obf[:, j * P:(j + 1) * P], ident)
            oT = oT_pool.tile([P, NCH, P], BF16)
            nc.vector.tensor_copy(oT, ot.rearrange("p (a b) -> p a b", a=NCH))
            zps = mm_ps.tile([P, C], FP32, tag="mm")
            for j in range(NCH):
                nc.tensor.matmul(zps, oT[:, j, :], w_bf["wp"][:, j, :],
                                 start=(j == 0), stop=(j == NCH - 1))

            # final: z + bfin + x  -> DRAM
            xr = xr_pool.tile([P, C], FP32)
            nc.sync.dma_start(xr, x_d[qi * P:(qi + 1) * P, :])
            res = res_pool.tile([P, C], FP32, tag="res")
            nc.vector.tensor_add(res, zps, bfin_bc)
            nc.gpsimd.tensor_add(res, res, xr)
            nc.sync.dma_start(out_d[qi * P:(qi + 1) * P, :], res)

        prev = None
        for qi in range(NQ) if "p3" not in ablate else []:
            cur = emit_scores_softmax(qi)
            if prev is not None:
                emit_attn_tail(prev)
            prev = cur
        if prev is not None:
            emit_attn_tail(prev)


_NC_CACHE = None


def _get_program():
    global _NC_CACHE
    if _NC_CACHE is None:
        _NC_CACHE = build_program()
    return _NC_CACHE


def kernel(x, gamma, beta, Wq, bq, Wk, bk, Wv, bv, Wp, bp):
    x = np.asarray(x, dtype=np.float32).reshape(B, HW, C)
    f32 = lambda a: np.ascontiguousarray(np.asarray(a, dtype=np.float32))
    row = lambda a: f32(a).reshape(1, C)
    nc = _get_program()
    in_maps = []
    for core in range(8):
        b, off = core // 2, (core % 2) * QH
        xb = x[b]
        x_roll = np.ascontiguousarray(np.concatenate([xb[off:], xb[:off]], axis=0))
        in_maps.append({
            "x": x_roll,
            "wq": f32(Wq), "wk": f32(Wk), "wv": f32(Wv), "wp": f32(Wp),
            "bq": row(bq), "bk": row(bk), "bv": row(bv), "bp": row(bp),
            "gamma": row(gamma), "beta": row(beta),
        })
    res = run_bass_kernel_spmd(nc, in_maps, core_ids=list(range(8)))
    out = np.empty((B, HW, C), np.float32)
    for core in range(8):
        b, off = core // 2, (core % 2) * QH
        out[b, off:off + QH] = res.results[core]["out"]
    return out.reshape(B, H, W, C)
